# revision 18
# baseline (speedup 1.0000x reference)
"""DeepFwFM (nn_DeepFwFM_12610023981507) Bass/Tile kernel for 8 TRN2 cores.

Self-contained: accepts the FULL unsharded inputs, shards the batch across 8
NeuronCores (data parallel; weights replicated), runs one SPMD Bass kernel,
and gathers the per-core outputs into the full [16384] result.

Math (per sample b):
  V[b, f, d]: 13 numeric fields (num_emb * Xv) + 26 gathered cat rows.
  h = V.reshape(390)
  first  = h . w                      (w = fwfm_w.flatten())
  second = h^T M h                    (M = 0.5*kron(S_offdiag, I10))
  deep   = W3 @ relu(W2 @ relu(W1 h + b1) + b2)
  y = first + second + deep + bias

The 390 h rows are embedded in a 512-row device space (4 chunks x 128, all
weights host-zero-padded; dead rows cost nothing since engine time scales
with the free dim only): rows 0..255 cat cols 0..255, 256..383 num cols
0..127, 384..385 num cols 128..129, 416..419 cat cols 256..259 (placed at
partition 32 of the tail chunk - engine APs may only start at partition
0/32/64/96). Per 512-sample block: ONE batched indirect DMA gathers all 26
categorical rows per sample (128x4x26 40B descriptors in a single Pool
instruction - SWDGE fixed cost ~1us/instruction makes instruction count the
lever) into a contiguous sample-major tile; PE transposes its 128-col
chunks into feature-major hT; numeric rows are produced feature-major
directly by a diag-expanded num_emb matmul against a host-pretransposed XvT
slice. Chunked fp32r PE matmuls (1 cycle/row at N=512 vs 4 for fp32)
compute U = M h, first, the MLP, and a ones-matmul partition-reduce of
E = U*h. ACT fuses relu+bias into the PSUM->SBUF copies. Every fp32r
matmul input is produced by an instruction with a float32r-rounded output
(BIR verifier requirement); DMA-loaded weights get a one-time DVE
round-copy, the per-block XvT slice a per-block one.
"""

import sys

for _p in ("/opt/trn_rl_repo", "/root/.axon_site/_ro/trn_rl_repo"):
    if _p not in sys.path:
        sys.path.append(_p)

import numpy as np

import concourse.bass as bass
import concourse.mybir as mybir
import concourse.tile as tile
from concourse import bacc
from concourse.bass_utils import run_bass_kernel_spmd
from concourse.masks import make_identity

P = 128
EMB = 10
NF = 39
NUM = 13
NCAT = 26
D_IN = NF * EMB  # 390
NCATC = NCAT * EMB  # 260
NNUMC = NUM * EMB  # 130
H1 = 400
KC = 4
DDEV = KC * P  # 512-row padded device h space
F32 = mybir.dt.float32
I32 = mybir.dt.int32
N_CORES = 8


def _dev_map():
    """dev_map[j] = original h index at device row j, or -1 if dead."""
    m = np.full(DDEV, -1, dtype=np.int64)
    m[0:256] = NNUMC + np.arange(256)       # cat cols 0..255
    m[256:384] = np.arange(128)             # num cols 0..127
    m[384:386] = [128, 129]                 # num cols 128..129
    m[416:420] = NNUMC + 256 + np.arange(4)  # cat cols 256..259
    return m


def host_prepare(inputs, n_cores):
    """Shard the batch; pack zero-padded permuted weights; gather offsets.

    cat_off[s, f] = vocab_offset[f] + Xi[s, f] - exact int32 row indices
    into the [rows, 40B] cat table (the indirect DMA's coef scales rows to
    bytes), so the device gather is a single indirect DMA per block with no
    on-device index arithmetic.
    """
    Xi = np.asarray(inputs["Xi"])
    Xv = np.ascontiguousarray(np.asarray(inputs["Xv"], dtype=np.float32))
    cat_table = np.ascontiguousarray(np.asarray(inputs["cat_table"], dtype=np.float32))
    num_emb = np.asarray(inputs["num_emb"], dtype=np.float32)
    offsets = np.asarray(inputs["offsets"], dtype=np.int64)
    fwfm_w = np.asarray(inputs["fwfm_w"], dtype=np.float32)
    field_cov_w = np.asarray(inputs["field_cov_w"], dtype=np.float32)
    W1 = np.asarray(inputs["W1"], dtype=np.float32)
    b1 = np.asarray(inputs["b1"], dtype=np.float32)
    W2 = np.asarray(inputs["W2"], dtype=np.float32)
    b2 = np.asarray(inputs["b2"], dtype=np.float32)
    W3 = np.asarray(inputs["W3"], dtype=np.float32)
    bias = np.asarray(inputs["bias"], dtype=np.float32)

    B = Xi.shape[0]
    assert B % n_cores == 0
    Bc = B // n_cores
    Xi2 = Xi.reshape(B, NCAT).astype(np.int64)

    # field sizes; split cat fields into two <=32768-row dma_gather groups
    # (int16 index limit) plus per-field indirect-DMA bigs.
    total_rows = cat_table.shape[0]
    sizes = np.diff(np.concatenate([offsets, [total_rows]])).astype(np.int64)
    smalls = [f for f in range(NCAT) if sizes[f] <= 32767]
    bigs = [f for f in range(NCAT) if sizes[f] > 32767]
    groupA, groupB, accA, accB = [], [], 0, 0
    for f in smalls:
        if accA + sizes[f] <= 32768:
            groupA.append(f); accA += sizes[f]
        else:
            assert accB + sizes[f] <= 32768, "small fields exceed two groups"
            groupB.append(f); accB += sizes[f]
    cat_perm = groupA + groupB + bigs  # G column order: A | B | bigs
    nA, nB, nBig = len(groupA), len(groupB), len(bigs)

    def packed_table(group):
        rows = int(sum(sizes[f] for f in group))
        t = np.zeros((rows, 64), dtype=np.float32)
        base, local_off = 0, {}
        for f in group:
            n = int(sizes[f])
            t[base:base + n, :EMB] = cat_table[offsets[f]:offsets[f] + n]
            local_off[f] = base
            base += n
        return t, local_off

    tabA, offA = packed_table(groupA)
    tabB, offB = packed_table(groupB)

    n_blocks_total = B // 512

    def wrapped_idx(group, loff):
        if not group:
            return np.zeros((n_blocks_total, 128, 0), np.int16)
        cols = np.stack(
            [Xi2[:, f] + loff[f] for f in group], axis=0
        ).astype(np.int16)                          # [nG, B]
        out = np.zeros((n_blocks_total, 16, len(group) * 32), dtype=np.int16)
        for blk in range(n_blocks_total):
            flat = cols[:, blk * 512:(blk + 1) * 512].reshape(-1)  # f-major
            i = np.arange(len(flat))
            out[blk, i % 16, i // 16] = flat
        return np.tile(out, (1, 8, 1)).reshape(n_blocks_total, 128, -1)

    idxA = wrapped_idx(groupA, offA)
    idxB = wrapped_idx(groupB, offB)
    # global row indices for big fields (indirect DMA, coef scales to bytes)
    big_rows = np.stack(
        [offsets[f] + Xi2[:, f] for f in bigs], axis=1
    ).astype(np.int32)                              # [B, nBig]

    dev = _dev_map()
    live = dev >= 0
    # original h col for G col j: cat field cat_perm[j//10], dim j%10
    catmap = np.array(
        [NNUMC + cat_perm[j // EMB] * EMB + (j % EMB) for j in range(NCATC)],
        dtype=np.int64)
    devo = dev.copy()
    iscat = (dev >= NNUMC)
    devo[iscat] = catmap[dev[iscat] - NNUMC]
    idx = devo[live]

    S = 0.5 * (field_cov_w + field_cov_w.T)
    Soff = S.copy()
    np.fill_diagonal(Soff, 0.0)
    M0 = 0.5 * np.kron(Soff, np.eye(EMB, dtype=np.float32))

    Md = np.zeros((DDEV, DDEV), dtype=np.float32)
    Md[np.ix_(live, live)] = M0[np.ix_(idx, idx)]
    W1d = np.zeros((DDEV, DDEV), dtype=np.float32)   # [k=dev h, m=dev h1]
    W1d[np.ix_(live, np.arange(H1))] = W1[:, idx].T
    W2d = np.zeros((DDEV, DDEV), dtype=np.float32)   # [k=dev h1, m=dev h2]
    W2d[:H1, :H1] = W2.T
    w3d = np.zeros((DDEV, 1), dtype=np.float32)
    w3d[:H1, 0] = W3[0]
    fwd = np.zeros((DDEV, 1), dtype=np.float32)
    fwd[live, 0] = fwfm_w.reshape(-1)[idx]
    onesd = np.ones((DDEV, 1), dtype=np.float32)

    def kchunk(mat):
        # [DDEV, cols] -> [P, KC, cols]
        return np.ascontiguousarray(
            mat.reshape(KC, P, -1).transpose(1, 0, 2))

    T_UF = kchunk(Md)
    W1T = kchunk(W1d)
    W2T = kchunk(W2d)
    w3 = kchunk(w3d)
    wfirst = kchunk(fwd)
    ones_pad = kchunk(onesd)

    def mchunk_vec(v):
        out = np.zeros((DDEV,), dtype=np.float32)
        out[:v.shape[0]] = v
        return np.ascontiguousarray(out.reshape(KC, P).T)

    b1_sb = mchunk_vec(b1)
    b2_sb = mchunk_vec(b2)
    bias_sb = bias.reshape(1, 1).astype(np.float32)

    # diag-expanded num_emb: EnumT[k, m] = num_emb[k, d(m)] * (f(m) == k)
    # for num cols m (m = f*10 + d); main chunk cols 0..127, tail 128..129.
    EnumT = np.zeros((NUM, P), dtype=np.float32)
    for m in range(P):
        EnumT[m // EMB, m] = num_emb[m // EMB, m % EMB]
    EnumTt = np.zeros((NUM, 2), dtype=np.float32)
    for t in range(2):
        m = P + t
        EnumTt[m // EMB, t] = num_emb[m // EMB, m % EMB]

    cat_u8 = cat_table.reshape(-1).view(np.uint8).reshape(-1, EMB * 4)

    shared = dict(
        cat_table=cat_u8, tabA=tabA, tabB=tabB,
        T_UF=T_UF, W1T=W1T, W2T=W2T, w3=w3,
        wfirst=wfirst, ones_pad=ones_pad, b1_sb=b1_sb, b2_sb=b2_sb,
        EnumT=EnumT, EnumTt=EnumTt, bias_sb=bias_sb,
    )
    nb = Bc // 512
    in_maps = []
    for c in range(n_cores):
        m = dict(shared)
        m["idxA"] = np.ascontiguousarray(
            idxA[c * nb:(c + 1) * nb].transpose(1, 0, 2))
        m["idxB"] = np.ascontiguousarray(
            idxB[c * nb:(c + 1) * nb].transpose(1, 0, 2))
        br = big_rows[c * Bc:(c + 1) * Bc]
        br = br.reshape(nb, 4, P, nBig).transpose(0, 2, 1, 3)
        m["big_rows"] = np.ascontiguousarray(br)   # [nb, P, KK, nBig]
        m["XvT"] = np.ascontiguousarray(Xv[c * Bc:(c + 1) * Bc, :NUM].T)
        in_maps.append(m)
    meta = dict(nA=nA, nB=nB, nBig=nBig)
    return in_maps, meta


def emit_dfm(tc, outs, ins, Bc, meta, dbg=False, repeat=1, hw_loop=True):
    """Emit the per-core kernel IR. outs/ins are dicts of DRAM APs."""
    nc = tc.nc
    SB = 512
    assert Bc % SB == 0
    n_blocks = Bc // SB
    KK = SB // P

    ct = ins["cat_table"]
    y = outs["y"]
    nA, nB, nBig = meta["nA"], meta["nB"], meta["nBig"]
    cA0 = 0
    cB0 = nA * EMB
    cG0 = (nA + nB) * EMB

    import contextlib

    def r32(ap):
        return ap.bitcast(mybir.dt.float32r)

    ctx = contextlib.ExitStack()
    with ctx:
        const = ctx.enter_context(tc.tile_pool(name="const", bufs=1))
        data = ctx.enter_context(tc.tile_pool(name="data", bufs=3))
        hpool = ctx.enter_context(tc.tile_pool(name="hpool", bufs=9))
        epool = ctx.enter_context(tc.tile_pool(name="epool", bufs=2))
        ypool = ctx.enter_context(tc.tile_pool(name="ypool", bufs=2))
        pt = ctx.enter_context(tc.tile_pool(name="pt", bufs=1, space="PSUM"))
        pmm = ctx.enter_context(tc.tile_pool(name="pmm", bufs=4, space="PSUM"))
        psm = ctx.enter_context(tc.tile_pool(name="psm", bufs=3, space="PSUM"))

        def load_const(name, shape, dtype=F32):
            t = const.tile(list(shape), dtype, tag=name)
            nc.sync.dma_start(t[:], ins[name][:])
            return t

        T_UF = load_const("T_UF", [P, KC, DDEV])
        W1T = load_const("W1T", [P, KC, DDEV])
        W2T = load_const("W2T", [P, KC, DDEV])
        w3 = load_const("w3", [P, KC, 1])
        wfirst = load_const("wfirst", [P, KC, 1])
        ones_sb = load_const("ones_pad", [P, KC, 1])
        EnumT = load_const("EnumT", [NUM, P])
        EnumTt = load_const("EnumTt", [NUM, 2])

        def round_const(t, shape, tag):
            tr = const.tile(list(shape), F32, tag=tag)
            nc.vector.tensor_copy(out=r32(tr[:]), in_=t[:])
            return tr

        T_UF = round_const(T_UF, [P, KC, DDEV], "T_UFr")
        W1T = round_const(W1T, [P, KC, DDEV], "W1Tr")
        W2T = round_const(W2T, [P, KC, DDEV], "W2Tr")
        w3 = round_const(w3, [P, KC, 1], "w3r")
        wfirst = round_const(wfirst, [P, KC, 1], "wfirstr")
        ones_sb = round_const(ones_sb, [P, KC, 1], "onesr")
        EnumT = round_const(EnumT, [NUM, P], "EnumTr")
        EnumTt = round_const(EnumTt, [NUM, 2], "EnumTtr")
        b1_sb = load_const("b1_sb", [P, KC])
        b2_sb = load_const("b2_sb", [P, KC])
        bias_sb = load_const("bias_sb", [1, 1])

        ident = const.tile([P, P], F32, tag="ident")
        make_identity(nc, ident[:])

        # persistent tail chunk: rows 0..1 num, 32..35 cat, rest zero
        hT3 = const.tile([P, SB], F32, tag="hT3")
        zscratch = const.tile([P, SB], F32, tag="zscratch")
        nc.vector.memset(zscratch[:], 0.0)
        nc.vector.tensor_copy(out=r32(hT3[:]), in_=zscratch[:])

        def block_body(bl):
            s0 = bl * SB
            bi = bl % n_blocks
            xvt_raw = data.tile([NUM, SB], F32, tag="xvtr")
            nc.sync.dma_start(xvt_raw[:], ins["XvT"][:, s0:s0 + SB])
            xvt = data.tile([NUM, SB], F32, tag="xvt")
            nc.vector.tensor_copy(out=r32(xvt[:]), in_=xvt_raw[:])

            G = data.tile([P, KK, NCATC], F32, tag="G")

            # smalls: one dma_gather per group from 256B-row side tables,
            # then repack the 10 live floats of each 64-float row into G
            for nm, nG, c0 in (("A", nA, cA0), ("B", nB, cB0)):
                if nG == 0:
                    continue
                ixt = data.tile([P, nG * 32], mybir.dt.int16, tag="ix" + nm)
                nc.sync.dma_start(ixt[:], ins["idx" + nm][:, bi, :])
                G256 = data.tile([P, KK * nG, 64], F32, tag="G" + nm)
                nc.gpsimd.dma_gather(
                    out_ap=G256[:],
                    in_ap=ins["tab" + nm][:, :],
                    idxs_ap=ixt[:],
                    num_idxs=nG * SB,
                    num_idxs_reg=nG * SB,
                    elem_size=64,
                    single_packet=False,
                )
                nc.vector.tensor_copy(
                    out=G[:, :, c0:c0 + nG * EMB].rearrange(
                        "p k (f d) -> p k f d", d=EMB),
                    in_=G256[:, :, :EMB].rearrange(
                        "p (f k) d -> p k f d", k=KK),
                )

            # bigs: per-(subtile, field) row-index indirect gathers
            if nBig:
                xi_sb = data.tile([P, KK, nBig], I32, tag="xi")
                nc.sync.dma_start(xi_sb[:], ins["big_rows"][bi])
                for kk in range(KK):
                    for j in range(nBig):
                        c0 = cG0 + j * EMB
                        nc.gpsimd.indirect_dma_start(
                            out=G[:, kk, c0:c0 + EMB].bitcast(mybir.dt.uint8),
                            out_offset=None,
                            in_=ct[:, :],
                            in_offset=bass.IndirectOffsetOnAxis(
                                ap=xi_sb[:, kk, j:j + 1], axis=0
                            ),
                            element_offset=0,
                        )

            # hT chunks 0,1: cat cols, PE-transposed per 128-sample subtile
            hT = []
            for c in range(2):
                dst = hpool.tile([P, SB], F32, tag="hT")
                for kk in range(KK):
                    ps = pt.tile([P, P], F32, tag="pt")
                    nc.tensor.transpose(
                        ps[:], G[:, kk, c * P:(c + 1) * P], ident[:]
                    )
                    nc.vector.tensor_copy(
                        out=r32(dst[:, kk * P:(kk + 1) * P]), in_=ps[:]
                    )
                hT.append(dst)

            # hT chunk 2: numeric cols 0..127, feature-major via one matmul
            psn = pmm.tile([P, SB], F32, tag="mm")
            nc.tensor.matmul(
                psn[:], lhsT=r32(EnumT[:]), rhs=r32(xvt[:]),
                start=True, stop=True,
            )
            dst2 = hpool.tile([P, SB], F32, tag="hT")
            nc.vector.tensor_copy(out=r32(dst2[:]), in_=psn[:])
            hT.append(dst2)

            # hT chunk 3: rows 0..1 = num cols 128..129; 32..35 = cat 256..259
            psn2 = pmm.tile([P, SB], F32, tag="mm")
            nc.tensor.matmul(
                psn2[:2, :], lhsT=r32(EnumTt[:]), rhs=r32(xvt[:]),
                start=True, stop=True,
            )
            nc.vector.tensor_copy(out=r32(hT3[:2, :]), in_=psn2[:2, :])
            for kk in range(KK):
                ps = pt.tile([P, P], F32, tag="pt")
                nc.tensor.transpose(
                    ps[:4, :], G[:, kk, 2 * P:2 * P + 4], ident[:]
                )
                nc.vector.tensor_copy(
                    out=r32(hT3[32:36, kk * P:(kk + 1) * P]), in_=ps[:4, :]
                )
            hT.append(hT3)

            acc = psm.tile([1, SB], F32, tag="small")
            for m in range(KC):
                ups = pmm.tile([P, SB], F32, tag="mm")
                for k in range(KC):
                    nc.tensor.matmul(
                        ups[:],
                        lhsT=r32(T_UF[:, k, m * P:(m + 1) * P]),
                        rhs=r32(hT[k][:, :]),
                        start=(k == 0),
                        stop=(k == KC - 1),
                    )
                Em = epool.tile([P, SB], F32, tag="E")
                nc.vector.tensor_tensor(
                    out=r32(Em[:]), in0=ups[:], in1=hT[m][:, :],
                    op=mybir.AluOpType.mult,
                )
                nc.tensor.matmul(
                    acc[:, :], lhsT=r32(ones_sb[:, m, :]), rhs=r32(Em[:]),
                    start=(m == 0), stop=False, skip_group_check=True,
                )
            for k in range(KC):
                nc.tensor.matmul(
                    acc[:, :], lhsT=r32(wfirst[:, k, :]), rhs=r32(hT[k][:, :]),
                    start=False, stop=False, skip_group_check=True,
                )

            h1 = []
            for m in range(KC):
                ps = pmm.tile([P, SB], F32, tag="mm")
                for k in range(KC):
                    nc.tensor.matmul(
                        ps[:],
                        lhsT=r32(W1T[:, k, m * P:(m + 1) * P]),
                        rhs=r32(hT[k][:, :]),
                        start=(k == 0),
                        stop=(k == KC - 1),
                    )
                dst = hpool.tile([P, SB], F32, tag="h1")
                nc.scalar.activation(
                    r32(dst[:]), ps[:],
                    mybir.ActivationFunctionType.Relu,
                    bias=b1_sb[:, m:m + 1],
                )
                h1.append(dst)

            h2 = []
            for m in range(KC):
                ps = pmm.tile([P, SB], F32, tag="mm")
                for k in range(KC):
                    nc.tensor.matmul(
                        ps[:],
                        lhsT=r32(W2T[:, k, m * P:(m + 1) * P]),
                        rhs=r32(h1[k][:]),
                        start=(k == 0),
                        stop=(k == KC - 1),
                    )
                dst = hpool.tile([P, SB], F32, tag="h2")
                nc.scalar.activation(
                    r32(dst[:]), ps[:],
                    mybir.ActivationFunctionType.Relu,
                    bias=b2_sb[:, m:m + 1],
                )
                h2.append(dst)

            for k in range(KC):
                nc.tensor.matmul(
                    acc[:, :], lhsT=r32(w3[:, k, :]), rhs=r32(h2[k][:]),
                    start=False, stop=(k == KC - 1), skip_group_check=True,
                )

            y_sb = ypool.tile([1, SB], F32, tag="y")
            nc.vector.tensor_scalar(
                out=y_sb[:], in0=acc[:], scalar1=bias_sb[:1, :1], scalar2=None,
                op0=mybir.AluOpType.add,
            )
            nc.sync.dma_start(out=y[None, s0:s0 + SB], in_=y_sb[:])
            if dbg:
                for nm, ps in (("dbg_sum", acc),):
                    t = ypool.tile([1, SB], F32, tag=nm)
                    nc.vector.tensor_copy(out=t[:], in_=ps[:])
                    nc.sync.dma_start(out=outs[nm][None, s0:s0 + SB], in_=t[:])

        if repeat > 1 and hw_loop:
            with tc.For_i(0, repeat) as _i:
                for bl in range(n_blocks):
                    block_body(bl)
        else:
            for bl in [b for _ in range(repeat) for b in range(n_blocks)]:
                block_body(bl)


def _build_module(in_map, Bc, meta, dbg=False, repeat=1, hw_loop=True):
    nc = bacc.Bacc(None, target_bir_lowering=False, debug=False,
                   num_devices=N_CORES)
    ins = {}
    dt_map = {np.dtype(np.int32): I32, np.dtype(np.float32): F32,
              np.dtype(np.uint8): mybir.dt.uint8,
              np.dtype(np.int16): mybir.dt.int16}
    for name, arr in in_map.items():
        ins[name] = nc.dram_tensor(
            name, list(arr.shape), dt_map[arr.dtype], kind="ExternalInput"
        ).ap()
    outs = {"y": nc.dram_tensor("y", [Bc], F32, kind="ExternalOutput").ap()}
    if dbg:
        for nm in ("dbg_first", "dbg_second", "dbg_deep"):
            outs[nm] = nc.dram_tensor(nm, [Bc], F32, kind="ExternalOutput").ap()
    with tile.TileContext(nc) as tc:
        emit_dfm(tc, outs, ins, Bc=Bc, meta=meta, dbg=dbg, repeat=repeat,
                 hw_loop=hw_loop)
    nc.compile()
    return nc


def run(inputs, trace=False, dbg=False, n_cores=None, **run_kwargs):
    """Run on 8 cores; returns (y_full, BassKernelResults)."""
    ncores = n_cores or N_CORES
    in_maps, meta = host_prepare(inputs, ncores)
    Bc = in_maps[0]["XvT"].shape[1]
    nc = _build_module(in_maps[0], Bc, meta, dbg=dbg)
    res = run_bass_kernel_spmd(
        nc, in_maps, core_ids=list(range(ncores)), trace=trace, **run_kwargs
    )
    y = np.concatenate([r["y"].reshape(-1) for r in res.results])
    return y.astype(np.float32), res


def kernel(**inputs):
    y, _ = run(inputs, trace=False)
    return y
